# revision 9
# baseline (speedup 1.0000x reference)
"""FFM cell kernel for Trainium2, 8 NeuronCores, batch-parallel.

Math: per batch element b,
    gated[t,m] = (x@W_pre + b_pre)[t,m] * sigmoid(x@W_gin + b_gin)[t,m]
    state[t,m,c] = sum_{s<=t} exp((a_m + i*b_c)*(t-s)) * gated[s,m]
    zm = [state.re, state.im] @ W_mix + b_mix
    out = LN(zm * sig(gout)) + skip * (1 - sig(gout))

The complex diagonal recurrence is decoupled into two *real* first-order
scans using angle addition (z is real):
    A[t,ch] = e^{a_m} A[t-1,ch] + cos(b_c t) z[t,m]
    B[t,ch] = e^{a_m} B[t-1,ch] + sin(b_c t) z[t,m]
    state_re = cos(b_c t) A + sin(b_c t) B
    state_im = sin(b_c t) A - cos(b_c t) B
mapped onto the DVE hardware scan (tensor_tensor_scan), channels (m,c)
on partitions, time on the free dim; tables host-precomputed.

Schedule (v3): the DVE chain is the critical path (~9.05us/group:
1 fused modulation + 2 scans + 2 fused products + 1 combine, using
[C|S] and [S|-C] concatenated tables on [128,2048] ops). Inputs arrive
as per-segment DMAs on 4 queues ordered by need; pre/gin matmuls are
k-outer so compute starts after the first xT chunk lands; the 8 zm mix
accumulators stay PSUM-resident (no stage C); at the last group the
mix matmuls and LN tails interleave per token tile.
Sharding: batch element -> core; everything replicated; no collectives.
"""

import numpy as np

B, T, D = 8, 1024, 512
TR, CTX, OUT = 64, 16, 512
EPS = 1e-6
NCH = TR * CTX   # 1024 scan channels
NG = NCH // 128  # 8 channel groups of 128 partitions
NT = T // 128    # 8 token tiles
KD = D // 128    # 4 contraction chunks over D

# bf16 blob column layout (offsets in columns of the [128, NWCOL] blob)
OFF_WPG = 0                   # [Wpre 4x64 | Wgin 4x64]
OFF_XT = OFF_WPG + 2 * KD * TR
OFF_EXPM = OFF_XT + KD * T
OFF_CS = OFF_EXPM + NCH       # [COS | SIN]            [128, 2T]
OFF_SCM = OFF_CS + 2 * T      # [SIN | -COS]           [128, 2T]
OFF_WGOUT = OFF_SCM + 2 * T
OFF_WSKIP = OFF_WGOUT + KD * OUT
OFF_WMRE = OFF_WSKIP + KD * OUT
OFF_WMIM = OFF_WMRE + NG * OUT
NWCOL = OFF_WMIM + NG * OUT

_cache = {}


def build_program(n_rep=1, with_bias=True):
    import concourse.bacc as bacc
    import concourse.tile as tile
    import concourse.mybir as mybir
    from concourse.alu_op_type import AluOpType as op

    f32 = mybir.dt.float32
    f32r = mybir.dt.float32r
    bf16 = mybir.dt.bfloat16
    AF = mybir.ActivationFunctionType

    wb = with_bias
    nc = bacc.Bacc("TRN2", target_bir_lowering=False, debug=False)

    def din(name, shape, dt=bf16):
        return nc.dram_tensor(name, shape, dt, kind="ExternalInput").ap()

    wpg_d = din("wpg", (128, 2 * KD * TR))
    xt_d = [din(f"xt{k}", (128, T)) for k in range(KD)]
    expm_d = din("expm", (128, NCH))
    cs_d = din("cs", (128, 2 * T))
    scm_d = din("scm", (128, 2 * T))
    wgs_d = din("wgs", (128, 2 * KD * OUT))
    wmre_d = din("wmre", (128, NG * OUT))
    wmim_d = din("wmim", (128, NG * OUT))
    dec_d = din("dec", (128, NG), f32)
    bpre_d = din("bpre", (TR, 1), f32)
    bgin_d = din("bgin", (TR, 1), f32)
    bgout_d = din("bgout", (1, OUT), f32r)
    bskip_d = din("bskip", (1, OUT), f32r)
    bmix_d = din("bmix", (1, OUT), f32r)
    out_d = nc.dram_tensor("out", (T, OUT), bf16, kind="ExternalOutput").ap()

    with tile.TileContext(nc) as tc:
      for _rep in range(n_rep):
        with (
            tc.tile_pool(name="singles", bufs=1) as singles,
            tc.tile_pool(name="states", bufs=1) as states,
        ):
            # ---- input DMAs, 4 queues, ordered by first use ----
            wsb = singles.tile([128, NWCOL], bf16, tag="wsb", name="wsb")

            def seg(o, n):
                return wsb[:, o:o + n]
            DEC_sb = singles.tile([128, NG], f32, tag="dec", name="dec")
            # sync: xt0, xt3, wgs | scalar: wpg, xt1, scm, wmre
            # gpsimd: xt2, expm, cs, dec, wmim   (ordered by first use)
            nc.sync.dma_start(out=seg(OFF_XT + 0 * T, T), in_=xt_d[0])
            nc.scalar.dma_start(out=seg(OFF_WPG, 2 * KD * TR), in_=wpg_d)
            nc.gpsimd.dma_start(out=seg(OFF_XT + 2 * T, T), in_=xt_d[2])
            nc.sync.dma_start(out=seg(OFF_XT + 3 * T, T), in_=xt_d[3])
            nc.scalar.dma_start(out=seg(OFF_XT + 1 * T, T), in_=xt_d[1])
            nc.gpsimd.dma_start(out=seg(OFF_EXPM, NCH), in_=expm_d)
            nc.gpsimd.dma_start(out=seg(OFF_CS, 2 * T), in_=cs_d)
            nc.scalar.dma_start(out=seg(OFF_SCM, 2 * T), in_=scm_d)
            nc.gpsimd.dma_start(out=DEC_sb, in_=dec_d)
            nc.sync.dma_start(out=seg(OFF_WGOUT, 2 * KD * OUT), in_=wgs_d)
            nc.scalar.dma_start(out=seg(OFF_WMRE, NG * OUT), in_=wmre_d)
            nc.gpsimd.dma_start(out=seg(OFF_WMIM, NG * OUT), in_=wmim_d)

            Wpre_sb = [seg(OFF_WPG + k * TR, TR) for k in range(KD)]
            Wgin_sb = [seg(OFF_WPG + KD * TR + k * TR, TR) for k in range(KD)]
            xT_sb = [seg(OFF_XT + k * T, T) for k in range(KD)]
            EXPM_sb = seg(OFF_EXPM, NCH)
            CS_sb = seg(OFF_CS, 2 * T)
            SCm_sb = seg(OFF_SCM, 2 * T)
            Wgout_sb = [seg(OFF_WGOUT + k * OUT, OUT) for k in range(KD)]
            Wskip_sb = [seg(OFF_WSKIP + k * OUT, OUT) for k in range(KD)]
            Wmre_sb = [seg(OFF_WMRE + g * OUT, OUT) for g in range(NG)]
            Wmim_sb = [seg(OFF_WMIM + g * OUT, OUT) for g in range(NG)]

            eps_sb = singles.tile([128, 1], f32, tag="eps")
            nc.vector.memset(eps_sb, EPS)
            warm = singles.tile([128, 640], bf16, tag="warm")
            nc.vector.memset(warm, 0.0)
            if wb:
                bpre_sb = singles.tile([TR, 1], f32, tag="bpre")
                nc.sync.dma_start(out=bpre_sb, in_=bpre_d)
                bgin_sb = singles.tile([TR, 1], f32, tag="bgin")
                nc.sync.dma_start(out=bgin_sb, in_=bgin_d)
                bgout_sb = singles.tile([1, OUT], f32r, tag="bgout")
                nc.sync.dma_start(out=bgout_sb, in_=bgout_d)
                bskip_sb = singles.tile([1, OUT], f32r, tag="bskip")
                nc.sync.dma_start(out=bskip_sb, in_=bskip_d)
                bmix_sb = singles.tile([1, OUT], f32r, tag="bmix")
                nc.sync.dma_start(out=bmix_sb, in_=bmix_d)
                ones_sb = singles.tile([1, 128], f32, tag="ones")
                nc.vector.memset(ones_sb, 1.0)

            # persistent state tiles
            zxs = [states.tile([128, T], bf16, tag=f"zxs{g}", name=f"zxs{g}")
                   for g in range(NG)]
            srm = [states.tile([128, 2 * T], bf16, tag=f"srm{g}",
                               name=f"srm{g}") for g in range(NG)]
            gsigs = [states.tile([128, OUT], bf16, tag=f"gsig{ti}",
                                 name=f"gsig{ti}") for ti in range(NT)]
            skips = [states.tile([128, OUT], bf16, tag=f"skip{ti}",
                                 name=f"skip{ti}") for ti in range(NT)]
            omgs = [states.tile([128, OUT], bf16, tag=f"omg{ti}",
                                name=f"omg{ti}") for ti in range(NT)]

            def rep2(ap2d):  # [128, N] -> [128, 2, N] stride-0 view
                return ap2d.unsqueeze(1).to_broadcast(
                    (128, 2, ap2d.shape[1]))

            # ---- phase 1: gated = (pre + bpre) * sig(gin + bgin) ----
            gated = singles.tile([TR, T], bf16, tag="gated")
            with (
                tc.tile_pool(name="psumA", bufs=1, space="PSUM") as psumA,
                tc.tile_pool(name="wkA", bufs=1) as wkA,
            ):
                pre_ps = psumA.tile([TR, T], f32, tag="pre")
                gin_ps = psumA.tile([TR, T], f32, tag="gin")
                wm_ps = psumA.tile([128, 512], f32, tag="warm")
                for i in range(9):
                    nc.tensor.matmul(wm_ps, warm[:, 0:128], warm[:, 128:640],
                                     start=(i == 0), stop=(i == 8))
                for k in range(KD):
                    for h in range(2):
                        cols = slice(h * 512, (h + 1) * 512)
                        nc.tensor.matmul(pre_ps[:, cols], Wpre_sb[k],
                                         xT_sb[k][:, cols],
                                         start=(k == 0), stop=(k == KD - 1),
                                         skip_group_check=True)
                    for h in range(2):
                        cols = slice(h * 512, (h + 1) * 512)
                        nc.tensor.matmul(gin_ps[:, cols], Wgin_sb[k],
                                         xT_sb[k][:, cols],
                                         start=(k == 0), stop=(k == KD - 1),
                                         skip_group_check=True)
                gsigA = wkA.tile([TR, T], f32, tag="gsigA")
                if wb:
                    nc.scalar.activation(gsigA, gin_ps, AF.Sigmoid,
                                         bias=bgin_sb, scale=1.0)
                else:
                    nc.scalar.activation(gsigA, gin_ps, AF.Sigmoid)
                nc.vector.scalar_tensor_tensor(
                    out=gated, in0=pre_ps,
                    scalar=(bpre_sb if wb else 0.0), in1=gsigA,
                    op0=op.add, op1=op.mult)

            # ---- phase 2a: zx expansion (feeds the DVE chain) ----
            with tc.tile_pool(name="psumZ", bufs=2, space="PSUM") as psumZ:
                for g in range(NG):
                    zx = psumZ.tile([128, T], f32, tag="zx", name="zx")
                    for h in range(2):
                        cols = slice(h * 512, (h + 1) * 512)
                        nc.tensor.matmul(
                            zx[:, cols],
                            EXPM_sb[0:TR, g * 128:(g + 1) * 128],
                            gated[:, cols], start=True, stop=True)
                    nc.scalar.activation(zxs[g], zx, AF.Copy)

            # ---- phase 2b: gout/skip matmuls (feed the LN tail) ----
            with tc.tile_pool(name="psumG", bufs=2, space="PSUM") as psumG:
                for ti in range(NT):
                    tcols = slice(ti * 128, (ti + 1) * 128)
                    gout_ps = psumG.tile([128, OUT], f32, tag="gout",
                                         name="gout_ps")
                    for k in range(KD):
                        nc.tensor.matmul(gout_ps, xT_sb[k][:, tcols],
                                         Wgout_sb[k], start=(k == 0),
                                         stop=(not wb and k == KD - 1))
                    if wb:
                        nc.tensor.matmul(gout_ps,
                                         ones_sb.bitcast(f32r),
                                         bgout_sb, start=False, stop=True)
                    nc.scalar.activation(gsigs[ti], gout_ps, AF.Sigmoid)
                    nc.scalar.activation(omgs[ti], gout_ps, AF.Sigmoid,
                                         bias=0.0, scale=-1.0)
                    skip_ps = psumG.tile([128, OUT], f32, tag="skp",
                                         name="skip_ps")
                    for k in range(KD):
                        nc.tensor.matmul(skip_ps, xT_sb[k][:, tcols],
                                         Wskip_sb[k], start=(k == 0),
                                         stop=(not wb and k == KD - 1))
                    if wb:
                        nc.tensor.matmul(skip_ps,
                                         ones_sb.bitcast(f32r),
                                         bskip_sb, start=False, stop=True)
                    nc.scalar.activation(skips[ti], skip_ps, AF.Copy)

            # ---- group loop: DVE-paced scans + mix accumulation ----
            with (
                tc.tile_pool(name="psumM", bufs=1, space="PSUM") as psumM,
                tc.tile_pool(name="wkB", bufs=2) as wkB,
                tc.tile_pool(name="wkC", bufs=2) as wkC,
            ):
                zms = [psumM.tile([128, OUT], f32, tag=f"zm{ti}",
                                  name=f"zm{ti}") for ti in range(NT)]
                hts = [wkC.tile([128, OUT], f32, tag=f"h{ti}",
                                name=f"h{ti}") for ti in range(NT)]
                mvs = [wkC.tile([128, 2], f32, tag=f"mv{ti}",
                                name=f"mv{ti}") for ti in range(NT)]
                sds = [wkC.tile([128, 1], f32, tag=f"sd{ti}",
                                name=f"sd{ti}") for ti in range(NT)]
                rstds = [wkC.tile([128, 1], f32, tag=f"rs{ti}",
                                  name=f"rs{ti}") for ti in range(NT)]
                betas = [wkC.tile([128, 1], f32, tag=f"be{ti}",
                                  name=f"be{ti}") for ti in range(NT)]
                lns = [wkC.tile([128, OUT], bf16, tag=f"ln{ti}",
                                name=f"ln{ti}") for ti in range(NT)]

                for g in range(NG):
                    inAB = wkB.tile([128, 2 * T], bf16, tag="mod")
                    nc.vector.tensor_tensor(inAB, CS_sb, rep2(zxs[g]),
                                            op.mult)
                    ab = wkB.tile([128, 2 * T], bf16, tag="scn")
                    dec_b = DEC_sb[:, g:g + 1].to_broadcast((128, T))
                    nc.vector.tensor_tensor_scan(
                        ab[:, 0:T], dec_b, inAB[:, 0:T], 0.0,
                        op.mult, op.add)
                    nc.vector.tensor_tensor_scan(
                        ab[:, T:2 * T], dec_b, inAB[:, T:2 * T], 0.0,
                        op.mult, op.add)
                    X = wkB.tile([128, 2 * T], bf16, tag="mod")
                    nc.vector.tensor_tensor(X, CS_sb, rep2(ab[:, 0:T]),
                                            op.mult)
                    Y = wkB.tile([128, 2 * T], bf16, tag="scn2")
                    nc.vector.tensor_tensor(Y, SCm_sb, rep2(ab[:, T:2 * T]),
                                            op.mult)
                    nc.vector.tensor_tensor(srm[g], X, Y, op.add)
                    # mix accumulation for all 8 token tiles
                    for tj in range(NT):
                        tc2 = slice(tj * 128, (tj + 1) * 128)
                        tc2i = slice(T + tj * 128, T + (tj + 1) * 128)
                        nc.tensor.matmul(zms[tj], srm[g][:, tc2],
                                         Wmre_sb[g], start=(g == 0),
                                         stop=False, skip_group_check=True)
                        nc.tensor.matmul(zms[tj], srm[g][:, tc2i],
                                         Wmim_sb[g], start=False,
                                         stop=(not wb and g == NG - 1),
                                         skip_group_check=True)
                        if g == NG - 1:
                            if wb:
                                nc.tensor.matmul(zms[tj],
                                                 ones_sb.bitcast(f32r),
                                                 bmix_sb, start=False,
                                                 stop=True,
                                                 skip_group_check=True)
                            h_t = hts[tj]
                            nc.vector.tensor_tensor(h_t, gsigs[tj], zms[tj],
                                                    op.mult)
                            stats = wkC.tile([128, 6], f32, tag="stats",
                                             name="stats")
                            nc.vector.bn_stats(stats, h_t)
                            nc.vector.bn_aggr(mvs[tj], stats)
                            nc.scalar.activation(sds[tj], mvs[tj][:, 1:2],
                                                 AF.Sqrt, bias=eps_sb,
                                                 scale=1.0)
                if True:
                    for tj in range(NT):
                        nc.vector.reciprocal(rstds[tj], sds[tj])
                        nc.vector.scalar_tensor_tensor(
                            out=betas[tj], in0=mvs[tj][:, 0:1], scalar=-1.0,
                            in1=rstds[tj], op0=op.mult, op1=op.mult)
                        nc.scalar.activation(lns[tj], hts[tj], AF.Identity,
                                             bias=betas[tj], scale=rstds[tj])
                    for tj in range(NT):
                        sk2 = wkC.tile([128, OUT], bf16, tag="sk2",
                                       name="sk2")
                        nc.vector.tensor_tensor(sk2, omgs[tj], skips[tj],
                                                op.mult)
                        outt = wkC.tile([128, OUT], bf16, tag="outt",
                                        name="outt")
                        nc.vector.tensor_tensor(outt, lns[tj], sk2, op.add)
                        nc.sync.dma_start(
                            out=out_d[tj * 128:(tj + 1) * 128, :], in_=outt)

    nc.compile()
    return nc


def host_prep(inputs):
    """Compute per-core input maps from the full problem inputs."""
    import ml_dtypes
    bfl = ml_dtypes.bfloat16

    x = np.asarray(inputs["x"], np.float32)
    a = np.abs(np.asarray(inputs["ffa_a"], np.float64))       # [TR]
    b = np.asarray(inputs["ffa_b"], np.float64)               # [CTX]
    t = np.arange(T, dtype=np.float64)

    cos_cols = np.cos(b[:, None] * t[None, :])                # [CTX, T]
    sin_cols = np.sin(b[:, None] * t[None, :])
    COS = np.tile(cos_cols, (8, 1))                           # [128, T]
    SIN = np.tile(sin_cols, (8, 1))
    CS = np.concatenate([COS, SIN], axis=1).astype(bfl)
    SCm = np.concatenate([SIN, -COS], axis=1).astype(bfl)

    dec = np.exp(-a).astype(np.float32)                       # [TR]
    rr = np.arange(128)
    DEC = np.empty((128, NG), np.float32)
    for g in range(NG):
        DEC[:, g] = dec[8 * g + rr // 16]

    col = np.arange(NCH)
    EXPM = np.zeros((128, NCH), np.float32)
    EXPM[:TR] = (np.arange(TR)[:, None] == (col[None, :] // CTX))

    Wm = np.asarray(inputs["W_mix"], np.float32).reshape(TR, 2, CTX, OUT)
    Wmre = np.ascontiguousarray(Wm[:, 0].reshape(NCH, OUT))
    Wmim = np.ascontiguousarray(Wm[:, 1].reshape(NCH, OUT))

    def chunks128(arr):  # [D or NCH, N] -> [128, k*N] chunk-concat
        r, n = arr.shape
        k = r // 128
        return np.concatenate([arr[i * 128:(i + 1) * 128] for i in range(k)],
                              axis=1)

    Wpre = chunks128(np.asarray(inputs["W_pre"], np.float32))   # [128, 256]
    Wgin = chunks128(np.asarray(inputs["W_gin"], np.float32))
    Wgout = chunks128(np.asarray(inputs["W_gout"], np.float32))  # [128, 2048]
    Wskip = chunks128(np.asarray(inputs["W_skip"], np.float32))

    shared = {
        "wpg": np.concatenate([Wpre, Wgin], axis=1).astype(bfl),
        "expm": EXPM.astype(bfl),
        "cs": CS, "scm": SCm,
        "wgs": np.concatenate([Wgout, Wskip], axis=1).astype(bfl),
        "wmre": chunks128(Wmre).astype(bfl),
        "wmim": chunks128(Wmim).astype(bfl),
        "dec": DEC,
        "bpre": np.asarray(inputs["b_pre"], np.float32).reshape(TR, 1),
        "bgin": np.asarray(inputs["b_gin"], np.float32).reshape(TR, 1),
        "bgout": np.asarray(inputs["b_gout"], np.float32).reshape(1, OUT),
        "bskip": np.asarray(inputs["b_skip"], np.float32).reshape(1, OUT),
        "bmix": np.asarray(inputs["b_mix"], np.float32).reshape(1, OUT),
    }
    in_maps = []
    for core in range(B):
        m = dict(shared)
        xTc = np.ascontiguousarray(x[core].T).astype(bfl)     # [512, 1024]
        for k in range(KD):
            m[f"xt{k}"] = np.ascontiguousarray(xTc[k * 128:(k + 1) * 128])
        in_maps.append(m)
    return in_maps


def kernel(**inputs):
    from concourse import bass_utils

    wb = any(
        np.any(np.asarray(inputs[k]))
        for k in ("b_pre", "b_gin", "b_gout", "b_skip", "b_mix")
    )
    key = f"nc_wb{wb}"
    if key not in _cache:
        _cache[key] = build_program(with_bias=wb)
    nc = _cache[key]
    in_maps = host_prep(inputs)
    res = bass_utils.run_bass_kernel_spmd(nc, in_maps, core_ids=list(range(B)))
    return np.stack([np.asarray(res.results[i]["out"], np.float32)
                     for i in range(B)])
